# revision 1
# baseline (speedup 1.0000x reference)
import sys
sys.path.insert(0, '/opt/trn_rl_repo')
import numpy as np
import concourse.bass as bass
import concourse.bacc as bacc
import concourse.mybir as mybir
from concourse.tile import TileContext
from concourse.bass_utils import run_bass_kernel_spmd

F32 = mybir.dt.float32
F32R = mybir.dt.float32r
AF = mybir.ActivationFunctionType
ALU = mybir.AluOpType
EPS = 1e-5

B, C, D, HH, W = 2, 256, 32, 32, 32
S = D * HH * W            # 32768 spatial per batch
NCORES = 8
SHARDS = 4                # spatial shards per batch
T_TOT = S // SHARDS       # 8192 tokens per core
CHUNK = 512
NCH = T_TOT // CHUNK      # 16 chunks
CTX, CD = 77, 768
H, HD = 8, 32
SCALE = HD ** -0.5
GN_N = float(32 * S)      # elements per group (32 ch x full spatial)

_cache = {}


def _build():
    nc = bacc.Bacc("TRN2", target_bir_lowering=False, debug=False, num_devices=NCORES)
    dp = lambda n, s: nc.dram_tensor(n, s, F32, kind="ExternalInput").ap()
    x_d = dp("x", [C, T_TOT])
    ctx_d = dp("ctx", [CTX, CD])
    wq_d = dp("wq", [C, C])
    wk_d = dp("wk", [CD, C])
    wv_d = dp("wv", [CD, C])
    wo_d = dp("wo", [C, C])
    gg_d = dp("gg", [C, 1])    # gn gamma
    gb_d = dp("gb", [C, 1])    # gn beta
    lg_d = dp("lg", [CD, 1])   # ln gamma
    lb_d = dp("lb", [CD, 1])   # ln beta
    bd_d = dp("bd", [128, 128])  # 32-block-diagonal ones
    bm_d = dp("bm", [128, 2])    # batch mask: col b==mybatch -> 1 else 0
    id_d = dp("idm", [128, 128])  # identity
    out_d = nc.dram_tensor("out", [C, T_TOT], F32, kind="ExternalOutput").ap()
    gn_in = nc.dram_tensor("gn_in", [128, 8], F32)
    gn_out = nc.dram_tensor("gn_out", [128, 8], F32, addr_space="Shared")

    with TileContext(nc, num_cores=NCORES) as tc:
        with tc.tile_pool(name="const", bufs=1) as const, \
             tc.tile_pool(name="work", bufs=2) as work, \
             tc.tile_pool(name="psA", bufs=1, space="PSUM") as psA_pool, \
             tc.tile_pool(name="psB", bufs=1, space="PSUM") as psB_pool:

            # ---------- loads ----------
            x_sb = [const.tile([128, T_TOT], F32R, tag=f"x{i}", name=f"x{i}") for i in range(2)]
            for i in range(2):
                nc.sync.dma_start(out=x_sb[i][:], in_=x_d[i*128:(i+1)*128, :].bitcast(F32R))
            ctx_sb = const.tile([CTX, CD], F32)
            nc.sync.dma_start(out=ctx_sb[:], in_=ctx_d[:])
            wq_sb = [const.tile([128, C], F32, tag=f"wq{i}", name=f"wq{i}") for i in range(2)]
            wo_sb = [const.tile([128, C], F32R, tag=f"wo{i}", name=f"wo{i}") for i in range(2)]
            for i in range(2):
                nc.sync.dma_start(out=wq_sb[i][:], in_=wq_d[i*128:(i+1)*128, :])
                nc.sync.dma_start(out=wo_sb[i][:], in_=wo_d[i*128:(i+1)*128, :].bitcast(F32R))
            wk_sb = [const.tile([128, C], F32, tag=f"wk{i}", name=f"wk{i}") for i in range(6)]
            wv_sb = [const.tile([128, C], F32, tag=f"wv{i}", name=f"wv{i}") for i in range(6)]
            for i in range(6):
                nc.sync.dma_start(out=wk_sb[i][:], in_=wk_d[i*128:(i+1)*128, :])
                nc.sync.dma_start(out=wv_sb[i][:], in_=wv_d[i*128:(i+1)*128, :])
            gg_sb = [const.tile([128, 1], F32, tag=f"gg{i}", name=f"gg{i}") for i in range(2)]
            gb_sb = [const.tile([128, 1], F32, tag=f"gb{i}", name=f"gb{i}") for i in range(2)]
            for i in range(2):
                nc.sync.dma_start(out=gg_sb[i][:], in_=gg_d[i*128:(i+1)*128, :])
                nc.sync.dma_start(out=gb_sb[i][:], in_=gb_d[i*128:(i+1)*128, :])
            lg_sb = [const.tile([128, 1], F32, tag=f"lg{i}", name=f"lg{i}") for i in range(6)]
            lb_sb = [const.tile([128, 1], F32, tag=f"lb{i}", name=f"lb{i}") for i in range(6)]
            for i in range(6):
                nc.sync.dma_start(out=lg_sb[i][:], in_=lg_d[i*128:(i+1)*128, :])
                nc.sync.dma_start(out=lb_sb[i][:], in_=lb_d[i*128:(i+1)*128, :])
            bd_sb = const.tile([128, 128], F32)
            nc.sync.dma_start(out=bd_sb[:], in_=bd_d[:])
            bm_sb = const.tile([128, 2], F32)
            nc.sync.dma_start(out=bm_sb[:], in_=bm_d[:])
            ident = const.tile([128, 128], F32)
            nc.sync.dma_start(out=ident[:], in_=id_d[:])

            # ---------- GroupNorm stats (per-channel partials -> group sums) ----------
            partials = work.tile([128, 4], F32, tag="partials", name="partials")
            for i in range(2):
                st = work.tile([128, 16, 6], F32, tag="gnstats", name="gnstats")
                for j in range(16):
                    nc.vector.bn_stats(out=st[:, j, :], in_=x_sb[i][:, j*512:(j+1)*512].bitcast(F32))
                mv = work.tile([128, 2], F32, tag="gnmv", name="gnmv")
                nc.vector.bn_aggr(out=mv[:], in_=st[:])
                # S1 = mean * 8192 ; S2 = (var + mean^2) * 8192
                sq = work.tile([128, 1], F32, tag="gnsq", name="gnsq")
                nc.vector.tensor_mul(sq[:], mv[:, 0:1], mv[:, 0:1])
                nc.vector.tensor_add(sq[:], sq[:], mv[:, 1:2])
                nc.vector.tensor_scalar_mul(partials[:, 2*i:2*i+1], mv[:, 0:1], float(T_TOT))
                nc.vector.tensor_scalar_mul(partials[:, 2*i+1:2*i+2], sq[:], float(T_TOT))
            gps = psB_pool.tile([128, 4], F32, tag="big", name="gps", bufs=2)
            nc.tensor.matmul(gps[:], bd_sb[:], partials[:], start=True, stop=True)
            gsum = work.tile([128, 8], F32, tag="gsum", name="gsum")
            nc.vector.tensor_scalar_mul(gsum[:, 0:4], gps[:], bm_sb[:, 0:1])
            nc.vector.tensor_scalar_mul(gsum[:, 4:8], gps[:], bm_sb[:, 1:2])
            nc.sync.dma_start(out=gn_in[:], in_=gsum[:])
            nc.gpsimd.collective_compute(
                "AllReduce", ALU.add,
                replica_groups=[[0, 1, 2, 3, 4, 5, 6, 7]],
                ins=[gn_in[:]], outs=[gn_out[:]])
            gw = work.tile([128, 8], F32, tag="gw", name="gw")
            nc.sync.dma_start(out=gw[:], in_=gn_out[:])
            gs = work.tile([128, 4], F32, tag="gs", name="gs")
            nc.vector.tensor_scalar_mul(gs[:], gw[:, 0:4], bm_sb[:, 0:1])
            gs2 = work.tile([128, 4], F32, tag="gs2", name="gs2")
            nc.vector.tensor_scalar_mul(gs2[:], gw[:, 4:8], bm_sb[:, 1:2])
            nc.vector.tensor_add(gs[:], gs[:], gs2[:])

            eps_t = work.tile([128, 1], F32, tag="eps", name="eps")
            nc.vector.memset(eps_t[:], EPS)
            a_sb, b_sb = [], []
            for i in range(2):
                mu = work.tile([128, 1], F32, tag=f"mu{i}", name=f"mu{i}")
                nc.vector.tensor_scalar_mul(mu[:], gs[:, 2*i:2*i+1], 1.0 / GN_N)
                ms = work.tile([128, 1], F32, tag=f"ms{i}", name=f"ms{i}")
                nc.vector.tensor_scalar_mul(ms[:], gs[:, 2*i+1:2*i+2], 1.0 / GN_N)
                var = work.tile([128, 1], F32, tag=f"var{i}", name=f"var{i}")
                nc.vector.tensor_mul(var[:], mu[:], mu[:])
                nc.vector.tensor_sub(var[:], ms[:], var[:])
                std = work.tile([128, 1], F32, tag=f"std{i}", name=f"std{i}")
                nc.scalar.activation(out=std[:], in_=var[:], func=AF.Sqrt, bias=eps_t[:])
                rstd = work.tile([128, 1], F32, tag=f"rstd{i}", name=f"rstd{i}")
                nc.vector.reciprocal(out=rstd[:], in_=std[:])
                ai = const.tile([128, 1], F32, tag=f"ai{i}", name=f"ai{i}")
                nc.vector.tensor_mul(ai[:], rstd[:], gg_sb[i][:])
                bi = const.tile([128, 1], F32, tag=f"bi{i}", name=f"bi{i}")
                nc.vector.tensor_mul(bi[:], mu[:], ai[:])
                nc.vector.tensor_sub(bi[:], gb_sb[i][:], bi[:])
                a_sb.append(ai)
                b_sb.append(bi)

            # ---------- context layernorm ----------
            stats = work.tile([CTX, 3, 6], F32, tag="lnstats", name="lnstats")
            for i in range(3):
                nc.vector.bn_stats(out=stats[:, i, :], in_=ctx_sb[:, i*256:(i+1)*256])
            mvc = work.tile([CTX, 2], F32, tag="lnmv", name="lnmv")
            nc.vector.bn_aggr(out=mvc[:], in_=stats[:])
            stdc = work.tile([CTX, 1], F32, tag="lnstd", name="lnstd")
            nc.scalar.activation(out=stdc[:], in_=mvc[:, 1:2], func=AF.Sqrt, bias=eps_t[:CTX, :])
            rstdc = work.tile([CTX, 1], F32, tag="lnrstd", name="lnrstd")
            nc.vector.reciprocal(out=rstdc[:], in_=stdc[:])
            ctxn = work.tile([CTX, CD], F32, tag="ctxn", name="ctxn")
            nc.vector.tensor_scalar(out=ctxn[:], in0=ctx_sb[:], scalar1=mvc[:, 0:1],
                                    scalar2=rstdc[:], op0=ALU.subtract, op1=ALU.mult)

            # transpose ctxn -> 6 tiles [128, 77], fold ln gamma/beta
            ctxnT = [const.tile([128, CTX], F32, tag=f"cT{i}", name=f"cT{i}") for i in range(6)]
            for i in range(6):
                ps = psB_pool.tile([128, CTX], F32, tag="big", name="psT", bufs=2)
                nc.tensor.transpose(ps[:, :], ctxn[:, i*128:(i+1)*128], ident[:CTX, :CTX])
                nc.vector.tensor_scalar(out=ctxnT[i][:], in0=ps[:],
                                        scalar1=lg_sb[i][:], scalar2=lb_sb[i][:],
                                        op0=ALU.mult, op1=ALU.add)

            # kT [2 x (128, 77)] scaled by 1/sqrt(hd); f32r for scores lhsT
            kT = [const.tile([128, CTX], F32R, tag=f"kT{m}", name=f"kT{m}") for m in range(2)]
            for m in range(2):
                ps = psB_pool.tile([128, CTX], F32, tag="big", name="psK", bufs=2)
                for k in range(6):
                    nc.tensor.matmul(ps[:], wk_sb[k][:, m*128:(m+1)*128],
                                     ctxnT[k][:], start=(k == 0), stop=(k == 5))
                nc.vector.tensor_scalar_mul(kT[m][:], ps[:], SCALE)

            # v [77, 256]
            v_sb = const.tile([CTX, C], F32)
            psv = psB_pool.tile([CTX, C], F32, tag="big", name="psV", bufs=2)
            for k in range(6):
                nc.tensor.matmul(psv[:], ctxnT[k][:], wv_sb[k][:],
                                 start=(k == 0), stop=(k == 5))
            nc.vector.tensor_copy(v_sb[:], psv[:])

            # zero-padded V / ones lhsT tiles [77, 128]
            zpad = const.tile([CTX, 128], F32)
            nc.vector.memset(zpad[:], 0.0)
            onescol = const.tile([CTX, 32], F32)
            nc.vector.memset(onescol[:], 1.0)
            vpad, opad = [], []
            for g in range(2):
                vp = [const.tile([CTX, 128], F32R, tag=f"vp{g}{j}", name=f"vp{g}{j}") for j in range(4)]
                op = [const.tile([CTX, 128], F32R, tag=f"op{g}{j}", name=f"op{g}{j}") for j in range(4)]
                for j in range(4):
                    h = 4*g + j
                    nc.vector.tensor_copy(vp[j][:], zpad[:])
                    nc.vector.tensor_copy(vp[j][:, 32*j:32*(j+1)], v_sb[:, 32*h:32*(h+1)])
                    nc.vector.tensor_copy(op[j][:], zpad[:])
                    nc.vector.tensor_copy(op[j][:, 32*j:32*(j+1)], onescol[:])
                vpad.append(vp)
                opad.append(op)

            # Wqa = a * Wq rows (f32r); q0 = Wq^T b
            wqa = [const.tile([128, C], F32R, tag=f"wqa{i}", name=f"wqa{i}") for i in range(2)]
            for i in range(2):
                nc.vector.tensor_scalar_mul(wqa[i][:], wq_sb[i][:], a_sb[i][:])
            q0 = []
            for m in range(2):
                ps = psB_pool.tile([128, 1], F32, tag="big", name="psq0", bufs=2)
                for k in range(2):
                    nc.tensor.matmul(ps[:], wq_sb[k][:, m*128:(m+1)*128], b_sb[k][:],
                                     start=(k == 0), stop=(k == 1))
                q0m = const.tile([128, 1], F32, tag=f"q0_{m}", name=f"q0_{m}")
                nc.vector.tensor_copy(q0m[:], ps[:])
                q0.append(q0m)

            # ---------- main token loop ----------
            for t in range(NCH):
                t0 = t * CHUNK
                qT = []
                for m in range(2):
                    ps = psB_pool.tile([128, CHUNK], F32, tag="big", name="ps_qt", bufs=2)
                    for k in range(2):
                        nc.tensor.matmul(ps[:], wqa[k][:, m*128:(m+1)*128],
                                         x_sb[k][:, t0:t0+CHUNK], start=(k == 0), stop=(k == 1))
                    qTm = work.tile([128, CHUNK], F32R, tag=f"qT{m}", name=f"qT{m}")
                    nc.scalar.activation(out=qTm[:], in_=ps[:], func=AF.Identity, bias=q0[m][:])
                    qT.append(qTm)

                attn_n = []
                for g in range(2):
                    scor = psA_pool.tile([CTX, 4*CHUNK], F32, tag="scores", name="scor")
                    for j in range(4):
                        nc.tensor.matmul(scor[:, j*CHUNK:(j+1)*CHUNK],
                                         kT[g][32*j:32*(j+1), :],
                                         qT[g][32*j:32*(j+1), :],
                                         start=True, stop=True, tile_position=(32*j, 0))
                    exps = work.tile([CTX, 4*CHUNK], F32R, tag="exps", name="exps")
                    nc.scalar.activation(out=exps[:], in_=scor[:], func=AF.Exp)
                    attn_ps = psB_pool.tile([128, CHUNK], F32, tag="attn", name="attn_ps")
                    zb_ps = psB_pool.tile([128, CHUNK], F32, tag="zb", name="zb_ps")
                    for j in range(4):
                        nc.tensor.matmul(attn_ps[:], vpad[g][j][:], exps[:, j*CHUNK:(j+1)*CHUNK],
                                         start=(j == 0), stop=(j == 3))
                        nc.tensor.matmul(zb_ps[:], opad[g][j][:], exps[:, j*CHUNK:(j+1)*CHUNK],
                                         start=(j == 0), stop=(j == 3))
                    zr = work.tile([128, CHUNK], F32, tag="zr", name="zr")
                    nc.vector.reciprocal(out=zr[:], in_=zb_ps[:])
                    anrm = work.tile([128, CHUNK], F32R, tag=f"anrm{g}", name=f"anrm{g}")
                    nc.vector.tensor_mul(anrm[:], attn_ps[:], zr[:])
                    attn_n.append(anrm)

                for m in range(2):
                    ps = psB_pool.tile([128, CHUNK], F32, tag="big", name="ps_out", bufs=2)
                    nc.tensor.matmul(ps[:], wo_sb[0][:, m*128:(m+1)*128],
                                     attn_n[0][:], start=True, stop=False)
                    nc.tensor.matmul(ps[:], wo_sb[1][:, m*128:(m+1)*128],
                                     attn_n[1][:], start=False, stop=True)
                    o_sb = work.tile([128, CHUNK], F32, tag="o_sb", name="o_sb")
                    nc.vector.tensor_add(o_sb[:], ps[:], x_sb[m][:, t0:t0+CHUNK].bitcast(F32))
                    nc.sync.dma_start(out=out_d[m*128:(m+1)*128, t0:t0+CHUNK], in_=o_sb[:])

    nc.compile()
    return nc


def _get_nc():
    if "nc" not in _cache:
        _cache["nc"] = _build()
    return _cache["nc"]


def kernel(x, context, gn_gamma, gn_beta, ln_gamma, ln_beta, Wq, Wk, Wv, Wo, bo,
           _trace=False):
    nc = _get_nc()
    x = np.asarray(x, dtype=np.float32)
    xr = x.reshape(B, C, S)
    bd = np.kron(np.eye(4, dtype=np.float32), np.ones((32, 32), np.float32))
    idm = np.eye(128, dtype=np.float32)
    col = lambda v: np.asarray(v, np.float32).reshape(-1, 1)
    common = {
        "wq": np.asarray(Wq, np.float32), "wk": np.asarray(Wk, np.float32),
        "wv": np.asarray(Wv, np.float32), "wo": np.asarray(Wo, np.float32),
        "gg": col(gn_gamma), "gb": col(gn_beta),
        "lg": col(ln_gamma), "lb": col(ln_beta),
        "bd": bd, "idm": idm,
    }
    in_maps = []
    for core in range(NCORES):
        b, s = core // SHARDS, core % SHARDS
        m = dict(common)
        m["x"] = np.ascontiguousarray(xr[b][:, s*T_TOT:(s+1)*T_TOT])
        bm = np.zeros((128, 2), np.float32); bm[:, b] = 1.0
        m["bm"] = bm
        m["ctx"] = np.ascontiguousarray(np.asarray(context, np.float32)[b])
        in_maps.append(m)
    res = run_bass_kernel_spmd(nc, in_maps, list(range(NCORES)), trace=_trace)
    out = np.empty((B, C, S), np.float32)
    for core in range(NCORES):
        b, s = core // SHARDS, core % SHARDS
        out[b][:, s*T_TOT:(s+1)*T_TOT] = res.results[core]["out"]
    out += np.asarray(bo, np.float32)[None, :, None]
    if _trace:
        _cache["last_exec_ns"] = res.exec_time_ns
        _cache["last_res"] = res
    return out.reshape(B, C, D, HH, W)



# revision 13
# speedup vs baseline: 28066.6714x; 28066.6714x over previous
import sys
sys.path.insert(0, '/opt/trn_rl_repo')
import numpy as np
import concourse.bass as bass
import concourse.bacc as bacc
import concourse.mybir as mybir
from concourse.tile import TileContext
from concourse.bass_utils import run_bass_kernel_spmd

F32 = mybir.dt.float32
F32R = mybir.dt.float32r
AF = mybir.ActivationFunctionType
ALU = mybir.AluOpType
EPS = 1e-5

B, C, D, HH, W = 2, 256, 32, 32, 32
S = D * HH * W            # 32768 spatial per batch
NCORES = 8
SHARDS = 4                # spatial shards per batch
T_TOT = S // SHARDS       # 8192 tokens per core
CHUNK = 512
NCH = T_TOT // CHUNK      # 16 chunks
NPC = 4                   # x DMA pieces per 128-channel half
PIECE = T_TOT // NPC
CTX, CD = 77, 768
H, HD = 8, 32
SCALE = HD ** -0.5
GN_N = float(32 * S)      # elements per group (32 ch x full spatial)

_cache = {}


def _build():
    nc = bacc.Bacc("TRN2", target_bir_lowering=False, debug=False, num_devices=NCORES)
    dp = lambda n, s: nc.dram_tensor(n, s, F32, kind="ExternalInput").ap()
    x_d = dp("x", [C, T_TOT])
    ctx_d = dp("ctx", [CTX, CD])
    wq_d = dp("wq", [C, C])
    wk_d = dp("wk", [CD, C])
    wv_d = dp("wv", [CD, C])
    wo_d = dp("wo", [C, C])
    gg_d = dp("gg", [C, 1])    # gn gamma
    gb_d = dp("gb", [C, 1])    # gn beta
    lg_d = dp("lg", [CD, 1])   # ln gamma
    lb_d = dp("lb", [CD, 1])   # ln beta
    bd_d = dp("bd", [128, 128])  # 32-block-diagonal ones
    id_d = dp("idm", [128, 128])  # identity
    out_d = nc.dram_tensor("out", [C, T_TOT], F32, kind="ExternalOutput").ap()

    bm_d = dp("bm", [128, 2])    # batch mask: col b==mybatch -> 1 else 0
    gn_in = nc.dram_tensor("gn_in", [128, 8], F32)
    gn_out = nc.dram_tensor("gn_out", [128, 8], F32, addr_space="Shared")

    with TileContext(nc, num_cores=NCORES) as tc:
        with tc.tile_pool(name="const", bufs=1) as const, \
             tc.tile_pool(name="work", bufs=2) as work, \
             tc.tile_pool(name="psS", bufs=1, space="PSUM") as psS, \
             tc.tile_pool(name="psQ", bufs=1, space="PSUM") as psQ, \
             tc.tile_pool(name="psAZ", bufs=1, space="PSUM") as psAZ, \
             tc.tile_pool(name="psO", bufs=1, space="PSUM") as psO:

            # ---------- input DMA ----------
            x_sb = [const.tile([128, T_TOT], F32R, tag=f"x{i}", name=f"x{i}") for i in range(2)]
            for p in range(NPC):
                sl = slice(p * PIECE, (p + 1) * PIECE)
                nc.sync.dma_start(out=x_sb[0][:, sl], in_=x_d[0:128, sl].bitcast(F32R))
                nc.sync.dma_start(out=x_sb[1][:, sl], in_=x_d[128:256, sl].bitcast(F32R))
            ctx_sb = const.tile([CTX, CD], F32)
            nc.sync.dma_start(out=ctx_sb[:], in_=ctx_d[:])
            wq_sb = [const.tile([128, C], F32, tag=f"wq{i}", name=f"wq{i}") for i in range(2)]
            wo_sb = [const.tile([128, C], F32R, tag=f"wo{i}", name=f"wo{i}") for i in range(2)]
            for i in range(2):
                nc.sync.dma_start(out=wq_sb[i][:], in_=wq_d[i*128:(i+1)*128, :])
                nc.sync.dma_start(out=wo_sb[i][:], in_=wo_d[i*128:(i+1)*128, :].bitcast(F32R))
            wk_sb = [const.tile([128, C], F32, tag=f"wk{i}", name=f"wk{i}") for i in range(6)]
            wv_sb = [const.tile([128, C], F32, tag=f"wv{i}", name=f"wv{i}") for i in range(6)]
            for i in range(6):
                nc.sync.dma_start(out=wk_sb[i][:], in_=wk_d[i*128:(i+1)*128, :])
                nc.sync.dma_start(out=wv_sb[i][:], in_=wv_d[i*128:(i+1)*128, :])
            gg_sb = [const.tile([128, 1], F32, tag=f"gg{i}", name=f"gg{i}") for i in range(2)]
            gb_sb = [const.tile([128, 1], F32, tag=f"gb{i}", name=f"gb{i}") for i in range(2)]
            for i in range(2):
                nc.sync.dma_start(out=gg_sb[i][:], in_=gg_d[i*128:(i+1)*128, :])
                nc.sync.dma_start(out=gb_sb[i][:], in_=gb_d[i*128:(i+1)*128, :])
            lg_sb = [const.tile([128, 1], F32, tag=f"lg{i}", name=f"lg{i}") for i in range(6)]
            lb_sb = [const.tile([128, 1], F32, tag=f"lb{i}", name=f"lb{i}") for i in range(6)]
            for i in range(6):
                nc.sync.dma_start(out=lg_sb[i][:], in_=lg_d[i*128:(i+1)*128, :])
                nc.sync.dma_start(out=lb_sb[i][:], in_=lb_d[i*128:(i+1)*128, :])
            bd_sb = const.tile([128, 128], F32)
            nc.sync.dma_start(out=bd_sb[:], in_=bd_d[:])
            bm_sb = const.tile([128, 2], F32)
            nc.sync.dma_start(out=bm_sb[:], in_=bm_d[:])
            ident = const.tile([128, 128], F32)
            nc.sync.dma_start(out=ident[:], in_=id_d[:])

            eps_t = work.tile([128, 1], F32, tag="eps", name="eps")
            nc.vector.memset(eps_t[:], EPS)

            # ---------- context layernorm + kT/v (independent of x) ----------
            stats = work.tile([CTX, 3, 6], F32, tag="lnstats", name="lnstats")
            for i in range(3):
                nc.vector.bn_stats(out=stats[:, i, :], in_=ctx_sb[:, i*256:(i+1)*256])
            mvc = work.tile([CTX, 2], F32, tag="lnmv", name="lnmv")
            nc.vector.bn_aggr(out=mvc[:], in_=stats[:])
            stdc = work.tile([CTX, 1], F32, tag="lnstd", name="lnstd")
            nc.scalar.activation(out=stdc[:], in_=mvc[:, 1:2], func=AF.Sqrt, bias=eps_t[:CTX, :])
            rstdc = work.tile([CTX, 1], F32, tag="lnrstd", name="lnrstd")
            nc.vector.reciprocal(out=rstdc[:], in_=stdc[:])
            ctxn = work.tile([CTX, CD], F32, tag="ctxn", name="ctxn")
            nc.vector.tensor_scalar(out=ctxn[:], in0=ctx_sb[:], scalar1=mvc[:, 0:1],
                                    scalar2=rstdc[:], op0=ALU.subtract, op1=ALU.mult)

            # transpose ctxn -> 6 tiles [128, 77], fold ln gamma/beta
            ctxnT = [const.tile([128, CTX], F32, tag=f"cT{i}", name=f"cT{i}") for i in range(6)]
            for i in range(6):
                ps = psAZ.tile([128, 512], F32, tag="az", name="psT", bufs=2)
                nc.tensor.transpose(ps[:, :CTX], ctxn[:, i*128:(i+1)*128], ident[:CTX, :CTX])
                nc.vector.tensor_scalar(out=ctxnT[i][:], in0=ps[:, :CTX],
                                        scalar1=lg_sb[i][:], scalar2=lb_sb[i][:],
                                        op0=ALU.mult, op1=ALU.add)

            # kT [2 x (128, 77)] scaled by 1/sqrt(hd); f32r for scores lhsT
            kT = [const.tile([128, CTX], F32R, tag=f"kT{m}", name=f"kT{m}") for m in range(2)]
            for m in range(2):
                ps = psAZ.tile([128, 512], F32, tag="az", name="psK", bufs=2)
                for k in range(6):
                    nc.tensor.matmul(ps[:, :CTX], wk_sb[k][:, m*128:(m+1)*128],
                                     ctxnT[k][:], start=(k == 0), stop=(k == 5))
                nc.vector.tensor_scalar_mul(kT[m][:], ps[:, :CTX], SCALE)

            # v [77, 256] (f32r, used as col-tiled lhsT slices)
            v_sb = const.tile([CTX, C], F32)
            psv = psAZ.tile([128, 512], F32, tag="az", name="psV", bufs=2)
            for k in range(6):
                nc.tensor.matmul(psv[:CTX, :C], ctxnT[k][:], wv_sb[k][:],
                                 start=(k == 0), stop=(k == 5))
            nc.vector.tensor_copy(v_sb[:], psv[:CTX, :C])

            zpad = const.tile([CTX, 128], F32)
            nc.vector.memset(zpad[:], 0.0)
            onescol = const.tile([CTX, 32], F32)
            nc.vector.memset(onescol[:], 1.0)
            vpad, opad = [], []
            for g in range(2):
                vp = [const.tile([CTX, 128], F32R, tag=f"vp{g}{j}", name=f"vp{g}{j}") for j in range(4)]
                op = [const.tile([CTX, 128], F32R, tag=f"op{g}{j}", name=f"op{g}{j}") for j in range(4)]
                for j in range(4):
                    h = 4*g + j
                    nc.vector.tensor_copy(vp[j][:], zpad[:])
                    nc.vector.tensor_copy(vp[j][:, 32*j:32*(j+1)], v_sb[:, 32*h:32*(h+1)].bitcast(F32))
                    nc.vector.tensor_copy(op[j][:], zpad[:])
                    nc.vector.tensor_copy(op[j][:, 32*j:32*(j+1)], onescol[:])
                vpad.append(vp)
                opad.append(op)

            # ---------- GroupNorm stats (pipelined with x DMA pieces) ----------
            partials_ch = work.tile([128, 4], F32, tag="pch", name="pch")
            for i in range(2):
                st = work.tile([128, 16, 6], F32, tag=f"gnstats{i}", name=f"gnstats{i}")
                for p in range(NPC):
                    for j in range(4):
                        blk = p * 4 + j
                        nc.vector.bn_stats(out=st[:, blk, :],
                                           in_=x_sb[i][:, blk*512:(blk+1)*512].bitcast(F32))
                mv = work.tile([128, 2], F32, tag=f"gnmv{i}", name=f"gnmv{i}")
                nc.vector.bn_aggr(out=mv[:], in_=st[:])
                # S1 = mean * 8192 ; S2 = (var + mean^2) * 8192
                sq = work.tile([128, 1], F32, tag=f"gnsq{i}", name=f"gnsq{i}")
                nc.vector.tensor_mul(sq[:], mv[:, 0:1], mv[:, 0:1])
                nc.vector.tensor_add(sq[:], sq[:], mv[:, 1:2])
                nc.vector.tensor_scalar_mul(partials_ch[:, 2*i:2*i+1], mv[:, 0:1], float(T_TOT))
                nc.vector.tensor_scalar_mul(partials_ch[:, 2*i+1:2*i+2], sq[:], float(T_TOT))
            gps = psO.tile([128, 4], F32, tag="o", name="gps", bufs=1)
            nc.tensor.matmul(gps[:], bd_sb[:], partials_ch[:], start=True, stop=True)
            gsum = work.tile([128, 8], F32, tag="gsum", name="gsum")
            nc.vector.tensor_scalar_mul(gsum[:, 0:4], gps[:], bm_sb[:, 0:1])
            nc.vector.tensor_scalar_mul(gsum[:, 4:8], gps[:], bm_sb[:, 1:2])
            nc.sync.dma_start(out=gn_in.ap()[:], in_=gsum[:])
            nc.gpsimd.collective_compute(
                "AllReduce", ALU.add,
                replica_groups=[[0, 1, 2, 3, 4, 5, 6, 7]],
                ins=[gn_in.ap()[:]], outs=[gn_out.ap()[:]])
            gw = work.tile([128, 8], F32, tag="gw", name="gw")
            nc.sync.dma_start(out=gw[:], in_=gn_out.ap()[:])
            gs = work.tile([128, 4], F32, tag="gs", name="gs")
            nc.vector.tensor_scalar_mul(gs[:], gw[:, 0:4], bm_sb[:, 0:1])
            gs2 = work.tile([128, 4], F32, tag="gs2", name="gs2")
            nc.vector.tensor_scalar_mul(gs2[:], gw[:, 4:8], bm_sb[:, 1:2])
            nc.vector.tensor_add(gs[:], gs[:], gs2[:])

            a_sb, b_sb = [], []
            for i in range(2):
                mu = work.tile([128, 1], F32, tag=f"mu{i}", name=f"mu{i}")
                nc.vector.tensor_scalar_mul(mu[:], gs[:, 2*i:2*i+1], 1.0 / GN_N)
                ms = work.tile([128, 1], F32, tag=f"ms{i}", name=f"ms{i}")
                nc.vector.tensor_scalar_mul(ms[:], gs[:, 2*i+1:2*i+2], 1.0 / GN_N)
                var = work.tile([128, 1], F32, tag=f"var{i}", name=f"var{i}")
                nc.vector.tensor_mul(var[:], mu[:], mu[:])
                nc.vector.tensor_sub(var[:], ms[:], var[:])
                std = work.tile([128, 1], F32, tag=f"std{i}", name=f"std{i}")
                nc.scalar.activation(out=std[:], in_=var[:], func=AF.Sqrt, bias=eps_t[:])
                rstd = work.tile([128, 1], F32, tag=f"rstd{i}", name=f"rstd{i}")
                nc.vector.reciprocal(out=rstd[:], in_=std[:])
                ai = const.tile([128, 1], F32, tag=f"ai{i}", name=f"ai{i}")
                nc.vector.tensor_mul(ai[:], rstd[:], gg_sb[i][:])
                bi = const.tile([128, 1], F32, tag=f"bi{i}", name=f"bi{i}")
                nc.vector.tensor_mul(bi[:], mu[:], ai[:])
                nc.vector.tensor_sub(bi[:], gb_sb[i][:], bi[:])
                a_sb.append(ai)
                b_sb.append(bi)

            # Wqa = a * Wq rows (f32r); q0 = Wq^T b
            wqa = [const.tile([128, C], F32R, tag=f"wqa{i}", name=f"wqa{i}") for i in range(2)]
            for i in range(2):
                nc.vector.tensor_scalar_mul(wqa[i][:], wq_sb[i][:], a_sb[i][:])
            q0 = []
            for m in range(2):
                ps = psQ.tile([128, CHUNK], F32, tag="q", name="psq0", bufs=1)
                for k in range(2):
                    nc.tensor.matmul(ps[:, 0:1], wq_sb[k][:, m*128:(m+1)*128], b_sb[k][:],
                                     start=(k == 0), stop=(k == 1))
                q0m = const.tile([128, 1], F32, tag=f"q0_{m}", name=f"q0_{m}")
                nc.vector.tensor_copy(q0m[:], ps[:, 0:1])
                q0.append(q0m)

            # ---------- main token loop ----------
            for t in range(NCH):
                t0 = t * CHUNK
                qT = []
                for m in range(2):
                    ps = psQ.tile([128, CHUNK], F32, tag="q", name="ps_qt", bufs=1)
                    for k in range(2):
                        nc.tensor.matmul(ps[:], wqa[k][:, m*128:(m+1)*128],
                                         x_sb[k][:, t0:t0+CHUNK], start=(k == 0), stop=(k == 1))
                    qTm = work.tile([128, CHUNK], F32R, tag=f"qT{m}", name=f"qT{m}")
                    nc.scalar.activation(out=qTm[:], in_=ps[:], func=AF.Identity, bias=q0[m][:])
                    qT.append(qTm)

                attn_n = []
                for g in range(2):
                    exhalf = []
                    for hh in range(2):
                        scor = psS.tile([CTX, 2*CHUNK], F32, tag="scor", name="scor", bufs=2)
                        for j2 in range(2):
                            j = 2*hh + j2
                            nc.tensor.matmul(scor[:, j2*CHUNK:(j2+1)*CHUNK],
                                             kT[g][32*j:32*(j+1), :],
                                             qT[g][32*j:32*(j+1), :],
                                             start=True, stop=True, tile_position=(32*j, 0))
                        ex = work.tile([CTX, 2*CHUNK], F32R, tag="exps", name="exps", bufs=3)
                        nc.scalar.activation(out=ex[:], in_=scor[:], func=AF.Exp)
                        exhalf.append(ex)
                    attn_ps = psAZ.tile([128, CHUNK], F32, tag="az", name="attn_ps", bufs=2)
                    zb_ps = psAZ.tile([128, CHUNK], F32, tag="az", name="zb_ps", bufs=2)
                    for j in range(4):
                        ex = exhalf[j // 2]
                        sl = slice((j % 2) * CHUNK, (j % 2 + 1) * CHUNK)
                        nc.tensor.matmul(attn_ps[:], vpad[g][j][:], ex[:, sl],
                                         start=(j == 0), stop=(j == 3))
                        nc.tensor.matmul(zb_ps[:], opad[g][j][:], ex[:, sl],
                                         start=(j == 0), stop=(j == 3))
                    zr = work.tile([128, CHUNK], F32, tag="zr", name="zr")
                    nc.vector.reciprocal(out=zr[:], in_=zb_ps[:])
                    anrm = work.tile([128, CHUNK], F32R, tag=f"anrm{g}", name=f"anrm{g}")
                    nc.vector.tensor_mul(anrm[:], attn_ps[:], zr[:])
                    attn_n.append(anrm)

                for m in range(2):
                    ps = psO.tile([128, CHUNK], F32, tag="o", name="ps_out", bufs=1)
                    nc.tensor.matmul(ps[:], wo_sb[0][:, m*128:(m+1)*128],
                                     attn_n[0][:], start=True, stop=False)
                    nc.tensor.matmul(ps[:], wo_sb[1][:, m*128:(m+1)*128],
                                     attn_n[1][:], start=False, stop=True)
                    o_sb = work.tile([128, CHUNK], F32, tag="o_sb", name="o_sb")
                    nc.vector.tensor_add(o_sb[:], ps[:], x_sb[m][:, t0:t0+CHUNK].bitcast(F32))
                    nc.sync.dma_start(out=out_d[m*128:(m+1)*128, t0:t0+CHUNK], in_=o_sb[:])

    nc.compile()
    return nc


def _get_nc():
    if "nc" not in _cache:
        _cache["nc"] = _build()
    return _cache["nc"]


def _in_maps(x, context, gn_gamma, gn_beta, ln_gamma, ln_beta, Wq, Wk, Wv, Wo, bo):
    x = np.asarray(x, dtype=np.float32)
    xr = x.reshape(B, C, S)
    bd = np.kron(np.eye(4, dtype=np.float32), np.ones((32, 32), np.float32))
    idm = np.eye(128, dtype=np.float32)
    col = lambda v: np.asarray(v, np.float32).reshape(-1, 1)
    common = {
        "wq": np.asarray(Wq, np.float32), "wk": np.asarray(Wk, np.float32),
        "wv": np.asarray(Wv, np.float32), "wo": np.asarray(Wo, np.float32),
        "gg": col(gn_gamma), "gb": col(gn_beta),
        "lg": col(ln_gamma), "lb": col(ln_beta),
        "bd": bd, "idm": idm,
    }
    in_maps = []
    for core in range(NCORES):
        b, s = core // SHARDS, core % SHARDS
        m = dict(common)
        m["x"] = np.ascontiguousarray(xr[b][:, s*T_TOT:(s+1)*T_TOT])
        bm = np.zeros((128, 2), np.float32); bm[:, b] = 1.0
        m["bm"] = bm
        m["ctx"] = np.ascontiguousarray(np.asarray(context, np.float32)[b])
        in_maps.append(m)
    return in_maps


def kernel(x, context, gn_gamma, gn_beta, ln_gamma, ln_beta, Wq, Wk, Wv, Wo, bo,
           _trace=False):
    nc = _get_nc()
    in_maps = _in_maps(x, context, gn_gamma, gn_beta, ln_gamma, ln_beta,
                       Wq, Wk, Wv, Wo, bo)
    res = run_bass_kernel_spmd(nc, in_maps, list(range(NCORES)), trace=_trace)
    out = np.empty((B, C, S), np.float32)
    for core in range(NCORES):
        b, s = core // SHARDS, core % SHARDS
        out[b][:, s*T_TOT:(s+1)*T_TOT] = res.results[core]["out"]
    out += np.asarray(bo, np.float32)[None, :, None]
    if _trace:
        _cache["last_exec_ns"] = res.exec_time_ns
        _cache["last_res"] = res
    return out.reshape(B, C, D, HH, W)


# revision 16
# speedup vs baseline: 28411.7956x; 1.0123x over previous
import sys
sys.path.insert(0, '/opt/trn_rl_repo')
import numpy as np
import concourse.bass as bass
import concourse.bacc as bacc
import concourse.mybir as mybir
from concourse.tile import TileContext
from concourse.bass_utils import run_bass_kernel_spmd

F32 = mybir.dt.float32
F32R = mybir.dt.float32r
AF = mybir.ActivationFunctionType
ALU = mybir.AluOpType
EPS = 1e-5

B, C, D, HH, W = 2, 256, 32, 32, 32
S = D * HH * W            # 32768 spatial per batch
NCORES = 8
SHARDS = 4                # spatial shards per batch
T_TOT = S // SHARDS       # 8192 tokens per core
CHUNK = 512
NCH = T_TOT // CHUNK      # 16 chunks
NPC = 4                   # x DMA pieces per 128-channel half
PIECE = T_TOT // NPC
CTX, CD = 77, 768
H, HD = 8, 32
SCALE = HD ** -0.5
GN_N = float(32 * S)      # elements per group (32 ch x full spatial)

_cache = {}


def _build():
    nc = bacc.Bacc("TRN2", target_bir_lowering=False, debug=False, num_devices=NCORES)
    dp = lambda n, s: nc.dram_tensor(n, s, F32, kind="ExternalInput").ap()
    x_d = dp("x", [C, T_TOT])
    ctx_d = dp("ctx", [CTX, CD])
    wq_d = dp("wq", [C, C])
    wk_d = dp("wk", [CD, C])
    wv_d = dp("wv", [CD, C])
    wo_d = dp("wo", [C, C])
    gg_d = dp("gg", [C, 1])    # gn gamma
    gb_d = dp("gb", [C, 1])    # gn beta
    lg_d = dp("lg", [CD, 1])   # ln gamma
    lb_d = dp("lb", [CD, 1])   # ln beta
    bd_d = dp("bd", [128, 128])  # 32-block-diagonal ones
    id_d = dp("idm", [128, 128])  # identity
    out_d = nc.dram_tensor("out", [C, T_TOT], F32, kind="ExternalOutput").ap()

    bm_d = dp("bm", [128, 2])    # batch mask: col b==mybatch -> 1 else 0
    gn_in = nc.dram_tensor("gn_in", [128, 8], F32)
    gn_out = nc.dram_tensor("gn_out", [128, 8], F32, addr_space="Shared")

    with TileContext(nc, num_cores=NCORES) as tc:
        with tc.tile_pool(name="const", bufs=1) as const, \
             tc.tile_pool(name="work", bufs=2) as work, \
             tc.tile_pool(name="psS", bufs=1, space="PSUM") as psS, \
             tc.tile_pool(name="psQ", bufs=1, space="PSUM") as psQ, \
             tc.tile_pool(name="psAZ", bufs=1, space="PSUM") as psAZ, \
             tc.tile_pool(name="psO", bufs=1, space="PSUM") as psO:

            # ---------- input DMA ----------
            x_sb = [const.tile([128, T_TOT], F32R, tag=f"x{i}", name=f"x{i}") for i in range(2)]
            for p in range(NPC):
                sl = slice(p * PIECE, (p + 1) * PIECE)
                nc.sync.dma_start(out=x_sb[0][:, sl], in_=x_d[0:128, sl].bitcast(F32R))
                nc.sync.dma_start(out=x_sb[1][:, sl], in_=x_d[128:256, sl].bitcast(F32R))
            ctx_sb = const.tile([CTX, CD], F32)
            nc.sync.dma_start(out=ctx_sb[:], in_=ctx_d[:])
            wq_sb = [const.tile([128, C], F32, tag=f"wq{i}", name=f"wq{i}") for i in range(2)]
            wo_sb = [const.tile([128, C], F32R, tag=f"wo{i}", name=f"wo{i}") for i in range(2)]
            for i in range(2):
                nc.sync.dma_start(out=wq_sb[i][:], in_=wq_d[i*128:(i+1)*128, :])
                nc.sync.dma_start(out=wo_sb[i][:], in_=wo_d[i*128:(i+1)*128, :].bitcast(F32R))
            wk_sb = [const.tile([128, C], F32, tag=f"wk{i}", name=f"wk{i}") for i in range(6)]
            wv_sb = [const.tile([128, C], F32, tag=f"wv{i}", name=f"wv{i}") for i in range(6)]
            for i in range(6):
                nc.sync.dma_start(out=wk_sb[i][:], in_=wk_d[i*128:(i+1)*128, :])
                nc.sync.dma_start(out=wv_sb[i][:], in_=wv_d[i*128:(i+1)*128, :])
            gg_sb = [const.tile([128, 1], F32, tag=f"gg{i}", name=f"gg{i}") for i in range(2)]
            gb_sb = [const.tile([128, 1], F32, tag=f"gb{i}", name=f"gb{i}") for i in range(2)]
            for i in range(2):
                nc.sync.dma_start(out=gg_sb[i][:], in_=gg_d[i*128:(i+1)*128, :])
                nc.sync.dma_start(out=gb_sb[i][:], in_=gb_d[i*128:(i+1)*128, :])
            lg_sb = [const.tile([128, 1], F32, tag=f"lg{i}", name=f"lg{i}") for i in range(6)]
            lb_sb = [const.tile([128, 1], F32, tag=f"lb{i}", name=f"lb{i}") for i in range(6)]
            for i in range(6):
                nc.sync.dma_start(out=lg_sb[i][:], in_=lg_d[i*128:(i+1)*128, :])
                nc.sync.dma_start(out=lb_sb[i][:], in_=lb_d[i*128:(i+1)*128, :])
            bd_sb = const.tile([128, 128], F32)
            nc.sync.dma_start(out=bd_sb[:], in_=bd_d[:])
            bm_sb = const.tile([128, 2], F32)
            nc.sync.dma_start(out=bm_sb[:], in_=bm_d[:])
            ident = const.tile([128, 128], F32)
            nc.sync.dma_start(out=ident[:], in_=id_d[:])

            eps_t = work.tile([128, 1], F32, tag="eps", name="eps")
            nc.vector.memset(eps_t[:], EPS)

            # ---------- context layernorm + kT/v (independent of x) ----------
            stats = work.tile([CTX, 3, 6], F32, tag="lnstats", name="lnstats")
            for i in range(3):
                nc.vector.bn_stats(out=stats[:, i, :], in_=ctx_sb[:, i*256:(i+1)*256])
            mvc = work.tile([CTX, 2], F32, tag="lnmv", name="lnmv")
            nc.vector.bn_aggr(out=mvc[:], in_=stats[:])
            stdc = work.tile([CTX, 1], F32, tag="lnstd", name="lnstd")
            nc.scalar.activation(out=stdc[:], in_=mvc[:, 1:2], func=AF.Sqrt, bias=eps_t[:CTX, :])
            rstdc = work.tile([CTX, 1], F32, tag="lnrstd", name="lnrstd")
            nc.vector.reciprocal(out=rstdc[:], in_=stdc[:])
            ctxn = work.tile([CTX, CD], F32, tag="ctxn", name="ctxn")
            nc.vector.tensor_scalar(out=ctxn[:], in0=ctx_sb[:], scalar1=mvc[:, 0:1],
                                    scalar2=rstdc[:], op0=ALU.subtract, op1=ALU.mult)

            # transpose ctxn -> 6 tiles [128, 77], fold ln gamma/beta
            ctxnT = [const.tile([128, CTX], F32, tag=f"cT{i}", name=f"cT{i}") for i in range(6)]
            for i in range(6):
                ps = psAZ.tile([128, 512], F32, tag="az", name="psT", bufs=2)
                nc.tensor.transpose(ps[:, :CTX], ctxn[:, i*128:(i+1)*128], ident[:CTX, :CTX])
                nc.vector.tensor_scalar(out=ctxnT[i][:], in0=ps[:, :CTX],
                                        scalar1=lg_sb[i][:], scalar2=lb_sb[i][:],
                                        op0=ALU.mult, op1=ALU.add)

            # kT [2 x (128, 77)] scaled by 1/sqrt(hd); f32r for scores lhsT
            kT = [const.tile([128, CTX], F32R, tag=f"kT{m}", name=f"kT{m}") for m in range(2)]
            for m in range(2):
                ps = psAZ.tile([128, 512], F32, tag="az", name="psK", bufs=2)
                for k in range(6):
                    nc.tensor.matmul(ps[:, :CTX], wk_sb[k][:, m*128:(m+1)*128],
                                     ctxnT[k][:], start=(k == 0), stop=(k == 5))
                nc.vector.tensor_scalar_mul(kT[m][:], ps[:, :CTX], SCALE)

            # v [77, 256] (f32r, used as col-tiled lhsT slices)
            v_sb = const.tile([CTX, C], F32)
            psv = psAZ.tile([128, 512], F32, tag="az", name="psV", bufs=2)
            for k in range(6):
                nc.tensor.matmul(psv[:CTX, :C], ctxnT[k][:], wv_sb[k][:],
                                 start=(k == 0), stop=(k == 5))
            nc.vector.tensor_copy(v_sb[:], psv[:CTX, :C])

            zpad = const.tile([CTX, 128], F32)
            nc.vector.memset(zpad[:], 0.0)
            onescol = const.tile([CTX, 32], F32)
            nc.vector.memset(onescol[:], 1.0)
            vpad, opad = [], []
            for g in range(2):
                vp = [const.tile([CTX, 128], F32R, tag=f"vp{g}{j}", name=f"vp{g}{j}") for j in range(4)]
                op = [const.tile([CTX, 128], F32R, tag=f"op{g}{j}", name=f"op{g}{j}") for j in range(4)]
                for j in range(4):
                    h = 4*g + j
                    nc.vector.tensor_copy(vp[j][:], zpad[:])
                    nc.vector.tensor_copy(vp[j][:, 32*j:32*(j+1)], v_sb[:, 32*h:32*(h+1)].bitcast(F32))
                    nc.vector.tensor_copy(op[j][:], zpad[:])
                    nc.vector.tensor_copy(op[j][:, 32*j:32*(j+1)], onescol[:])
                vpad.append(vp)
                opad.append(op)

            # ---------- GroupNorm stats (pipelined with x DMA pieces) ----------
            partials_ch = work.tile([128, 4], F32, tag="pch", name="pch")
            for i in range(2):
                st = work.tile([128, 16, 6], F32, tag=f"gnstats{i}", name=f"gnstats{i}")
                for p in range(NPC):
                    for j in range(4):
                        blk = p * 4 + j
                        nc.vector.bn_stats(out=st[:, blk, :],
                                           in_=x_sb[i][:, blk*512:(blk+1)*512].bitcast(F32))
                mv = work.tile([128, 2], F32, tag=f"gnmv{i}", name=f"gnmv{i}")
                nc.vector.bn_aggr(out=mv[:], in_=st[:])
                # S1 = mean * 8192 ; S2 = (var + mean^2) * 8192
                sq = work.tile([128, 1], F32, tag=f"gnsq{i}", name=f"gnsq{i}")
                nc.vector.tensor_mul(sq[:], mv[:, 0:1], mv[:, 0:1])
                nc.vector.tensor_add(sq[:], sq[:], mv[:, 1:2])
                nc.vector.tensor_scalar_mul(partials_ch[:, 2*i:2*i+1], mv[:, 0:1], float(T_TOT))
                nc.vector.tensor_scalar_mul(partials_ch[:, 2*i+1:2*i+2], sq[:], float(T_TOT))
            gps = psO.tile([128, 4], F32, tag="o", name="gps", bufs=1)
            nc.tensor.matmul(gps[:], bd_sb[:], partials_ch[:], start=True, stop=True)
            gsum = work.tile([128, 8], F32, tag="gsum", name="gsum")
            nc.vector.tensor_scalar_mul(gsum[:, 0:4], gps[:], bm_sb[:, 0:1])
            nc.vector.tensor_scalar_mul(gsum[:, 4:8], gps[:], bm_sb[:, 1:2])
            nc.gpsimd.dma_start(out=gn_in.ap()[:], in_=gsum[:])
            nc.gpsimd.collective_compute(
                "AllReduce", ALU.add,
                replica_groups=[[0, 1, 2, 3, 4, 5, 6, 7]],
                ins=[gn_in.ap()[:]], outs=[gn_out.ap()[:]])
            gw = work.tile([128, 8], F32, tag="gw", name="gw")
            nc.gpsimd.dma_start(out=gw[:], in_=gn_out.ap()[:])
            gs = work.tile([128, 4], F32, tag="gs", name="gs")
            nc.vector.tensor_scalar_mul(gs[:], gw[:, 0:4], bm_sb[:, 0:1])
            gs2 = work.tile([128, 4], F32, tag="gs2", name="gs2")
            nc.vector.tensor_scalar_mul(gs2[:], gw[:, 4:8], bm_sb[:, 1:2])
            nc.vector.tensor_add(gs[:], gs[:], gs2[:])

            a_sb, b_sb = [], []
            for i in range(2):
                mu = work.tile([128, 1], F32, tag=f"mu{i}", name=f"mu{i}")
                nc.vector.tensor_scalar_mul(mu[:], gs[:, 2*i:2*i+1], 1.0 / GN_N)
                ms = work.tile([128, 1], F32, tag=f"ms{i}", name=f"ms{i}")
                nc.vector.tensor_scalar_mul(ms[:], gs[:, 2*i+1:2*i+2], 1.0 / GN_N)
                var = work.tile([128, 1], F32, tag=f"var{i}", name=f"var{i}")
                nc.vector.tensor_mul(var[:], mu[:], mu[:])
                nc.vector.tensor_sub(var[:], ms[:], var[:])
                std = work.tile([128, 1], F32, tag=f"std{i}", name=f"std{i}")
                nc.scalar.activation(out=std[:], in_=var[:], func=AF.Sqrt, bias=eps_t[:])
                rstd = work.tile([128, 1], F32, tag=f"rstd{i}", name=f"rstd{i}")
                nc.vector.reciprocal(out=rstd[:], in_=std[:])
                ai = const.tile([128, 1], F32, tag=f"ai{i}", name=f"ai{i}")
                nc.vector.tensor_mul(ai[:], rstd[:], gg_sb[i][:])
                bi = const.tile([128, 1], F32, tag=f"bi{i}", name=f"bi{i}")
                nc.vector.tensor_mul(bi[:], mu[:], ai[:])
                nc.vector.tensor_sub(bi[:], gb_sb[i][:], bi[:])
                a_sb.append(ai)
                b_sb.append(bi)

            # Wqa = a * Wq rows (f32r); q0 = Wq^T b
            wqa = [const.tile([128, C], F32R, tag=f"wqa{i}", name=f"wqa{i}") for i in range(2)]
            for i in range(2):
                nc.vector.tensor_scalar_mul(wqa[i][:], wq_sb[i][:], a_sb[i][:])
            q0 = []
            for m in range(2):
                ps = psQ.tile([128, CHUNK], F32, tag="q", name="psq0", bufs=1)
                for k in range(2):
                    nc.tensor.matmul(ps[:, 0:1], wq_sb[k][:, m*128:(m+1)*128], b_sb[k][:],
                                     start=(k == 0), stop=(k == 1))
                q0m = const.tile([128, 1], F32, tag=f"q0_{m}", name=f"q0_{m}")
                nc.vector.tensor_copy(q0m[:], ps[:, 0:1])
                q0.append(q0m)

            # ---------- main token loop ----------
            for t in range(NCH):
                t0 = t * CHUNK
                qT = []
                for m in range(2):
                    ps = psQ.tile([128, CHUNK], F32, tag="q", name="ps_qt", bufs=1)
                    for k in range(2):
                        nc.tensor.matmul(ps[:], wqa[k][:, m*128:(m+1)*128],
                                         x_sb[k][:, t0:t0+CHUNK], start=(k == 0), stop=(k == 1))
                    qTm = work.tile([128, CHUNK], F32R, tag=f"qT{m}", name=f"qT{m}")
                    nc.scalar.activation(out=qTm[:], in_=ps[:], func=AF.Identity, bias=q0[m][:])
                    qT.append(qTm)

                attn_n = []
                for g in range(2):
                    exhalf = []
                    for hh in range(2):
                        scor = psS.tile([CTX, 2*CHUNK], F32, tag="scor", name="scor", bufs=2)
                        for j2 in range(2):
                            j = 2*hh + j2
                            nc.tensor.matmul(scor[:, j2*CHUNK:(j2+1)*CHUNK],
                                             kT[g][32*j:32*(j+1), :],
                                             qT[g][32*j:32*(j+1), :],
                                             start=True, stop=True, tile_position=(32*j, 0))
                        ex = work.tile([CTX, 2*CHUNK], F32R, tag="exps", name="exps", bufs=3)
                        nc.scalar.activation(out=ex[:], in_=scor[:], func=AF.Exp)
                        exhalf.append(ex)
                    attn_ps = psAZ.tile([128, CHUNK], F32, tag="az", name="attn_ps", bufs=2)
                    zb_ps = psAZ.tile([128, CHUNK], F32, tag="az", name="zb_ps", bufs=2)
                    for j in range(4):
                        ex = exhalf[j // 2]
                        sl = slice((j % 2) * CHUNK, (j % 2 + 1) * CHUNK)
                        nc.tensor.matmul(attn_ps[:], vpad[g][j][:], ex[:, sl],
                                         start=(j == 0), stop=(j == 3))
                        nc.tensor.matmul(zb_ps[:], opad[g][j][:], ex[:, sl],
                                         start=(j == 0), stop=(j == 3))
                    zr = work.tile([128, CHUNK], F32, tag="zr", name="zr")
                    nc.vector.reciprocal(out=zr[:], in_=zb_ps[:])
                    anrm = work.tile([128, CHUNK], F32R, tag=f"anrm{g}", name=f"anrm{g}")
                    nc.vector.tensor_mul(anrm[:], attn_ps[:], zr[:])
                    attn_n.append(anrm)

                for m in range(2):
                    ps = psO.tile([128, CHUNK], F32, tag="o", name="ps_out", bufs=1)
                    nc.tensor.matmul(ps[:], wo_sb[0][:, m*128:(m+1)*128],
                                     attn_n[0][:], start=True, stop=False)
                    nc.tensor.matmul(ps[:], wo_sb[1][:, m*128:(m+1)*128],
                                     attn_n[1][:], start=False, stop=True)
                    o_sb = work.tile([128, CHUNK], F32, tag="o_sb", name="o_sb")
                    nc.vector.tensor_add(o_sb[:], ps[:], x_sb[m][:, t0:t0+CHUNK].bitcast(F32))
                    nc.sync.dma_start(out=out_d[m*128:(m+1)*128, t0:t0+CHUNK], in_=o_sb[:])

    nc.compile()
    return nc


def _get_nc():
    if "nc" not in _cache:
        _cache["nc"] = _build()
    return _cache["nc"]


def _in_maps(x, context, gn_gamma, gn_beta, ln_gamma, ln_beta, Wq, Wk, Wv, Wo, bo):
    x = np.asarray(x, dtype=np.float32)
    xr = x.reshape(B, C, S)
    bd = np.kron(np.eye(4, dtype=np.float32), np.ones((32, 32), np.float32))
    idm = np.eye(128, dtype=np.float32)
    col = lambda v: np.asarray(v, np.float32).reshape(-1, 1)
    common = {
        "wq": np.asarray(Wq, np.float32), "wk": np.asarray(Wk, np.float32),
        "wv": np.asarray(Wv, np.float32), "wo": np.asarray(Wo, np.float32),
        "gg": col(gn_gamma), "gb": col(gn_beta),
        "lg": col(ln_gamma), "lb": col(ln_beta),
        "bd": bd, "idm": idm,
    }
    in_maps = []
    for core in range(NCORES):
        b, s = core // SHARDS, core % SHARDS
        m = dict(common)
        m["x"] = np.ascontiguousarray(xr[b][:, s*T_TOT:(s+1)*T_TOT])
        bm = np.zeros((128, 2), np.float32); bm[:, b] = 1.0
        m["bm"] = bm
        m["ctx"] = np.ascontiguousarray(np.asarray(context, np.float32)[b])
        in_maps.append(m)
    return in_maps


def kernel(x, context, gn_gamma, gn_beta, ln_gamma, ln_beta, Wq, Wk, Wv, Wo, bo,
           _trace=False):
    nc = _get_nc()
    in_maps = _in_maps(x, context, gn_gamma, gn_beta, ln_gamma, ln_beta,
                       Wq, Wk, Wv, Wo, bo)
    res = run_bass_kernel_spmd(nc, in_maps, list(range(NCORES)), trace=_trace)
    out = np.empty((B, C, S), np.float32)
    for core in range(NCORES):
        b, s = core // SHARDS, core % SHARDS
        out[b][:, s*T_TOT:(s+1)*T_TOT] = res.results[core]["out"]
    out += np.asarray(bo, np.float32)[None, :, None]
    if _trace:
        _cache["last_exec_ns"] = res.exec_time_ns
        _cache["last_res"] = res
    return out.reshape(B, C, D, HH, W)
